# revision 8
# baseline (speedup 1.0000x reference)
"""Trainium2 Bass kernel for nn_DecoderLSTM (30-step decoder LSTM, npeds=8192,
hidden=256, embed=64), data-parallel over peds across 8 NeuronCores.

v2: DoubleRow-fp8 gates + instruction-count-minimized pipeline.

Layout strategy (per core, 1024 peds split into 2 pipelined halves of 512):
  - Everything "transposed": partitions = feature dims, free = peds.
  - h state kept twice: bf16 HB [128, ch0|ch1] for the LayerNorm stats path,
    fp8e4m3 TH [128, 2, 512] for the gates matmul (DoubleRow groups = the two
    128-row hidden chunks -> one matmul per gate chunk contracts all 256 h
    dims at the cost of one).  W_hh is fp8 interleaved to match.
  - dec path (64-dim spatial embedding) stays bf16; its K=65 matmul carries a
    constant ones row that injects the (b_ih+b_hh) bias into PSUM, which lets
    activations fuse: sigmoid(i|f) in one [128,1024] ACT op per (half,chunk),
    tanh(c) in one [128,1024] per half.
  - LayerNorm2 folded algebraically: rel = sigmoid((A@h) * rsqrt(V) + b')
    with A = (g*pos_W) - rowsum(g*pos_W)/H, V = E[h^2] - mu^2 + eps (eps
    folded into the tail's DVE op, E[h^2] via one fp8-DoubleRow matmul over
    gpsimd-squared h).
  - Per-ped scalar tail runs 32x32 block-transposed, both halves at once
    ([64,16] ops).  rsqrt via int bit-trick seed + 1 Newton step.
  - LayerNorm1+embedding folded: ln1(p) = (s, -s), s = tanh(0.88/sqrt(4eps)
    * (rel0-rel1)); dec_in = prelu(s*w_emb + emb_b', 0.01) via one
    outer-product matmul + one Prelu activation per half.
  - One [64,512] f32->bf16 block-transpose yields the s row for the embed
    matmul AND the rel rows DMA'd to DRAM (output returned bf16, cast on
    host).
  - last_pos / lp carry is dead code (never affects output) -> dropped.
"""
import os
import sys

for _p in ("/root/.axon_site/_ro/trn_rl_repo", "/opt/trn_rl_repo"):
    if os.path.isdir(_p) and _p not in sys.path:
        sys.path.insert(0, _p)

import numpy as np
import ml_dtypes

import concourse.bass as bass
import concourse.tile as tile
from concourse import bacc, mybir
from concourse import bass_utils
from concourse.bass_interp import get_hw_module


def _ensure_ntff_hook_module():
    """Provide antenv.axon_hooks if the image ships without it, so
    run_bass_kernel_spmd(trace=True) can capture NTFF profiles."""
    try:
        from antenv import axon_hooks  # noqa: F401
        return
    except ImportError:
        pass
    import types

    mod = types.ModuleType("antenv.axon_hooks")
    mod._HOOK = None

    def set_axon_ntff_profile_hook(hook):
        mod._HOOK = hook

    def get_axon_ntff_profile_hook():
        if mod._HOOK is None:
            try:
                from trn_agent_boot.trn_boot import _ntff_profile_via_ctypes
                mod._HOOK = _ntff_profile_via_ctypes("/opt/axon/libaxon_pjrt.so")
            except Exception:
                mod._HOOK = None
        return mod._HOOK

    mod.set_axon_ntff_profile_hook = set_axon_ntff_profile_hook
    mod.get_axon_ntff_profile_hook = get_axon_ntff_profile_hook
    sys.modules["antenv.axon_hooks"] = mod
    try:
        import antenv
        antenv.axon_hooks = mod
    except ImportError:
        pass


_ensure_ntff_hook_module()

F32 = mybir.dt.float32
BF16 = mybir.dt.bfloat16
FP8 = mybir.dt.float8e4
DT = BF16
I32 = mybir.dt.int32
AF = mybir.ActivationFunctionType
OP = mybir.AluOpType
PM = mybir.MatmulPerfMode

N_CORES = 8
NPEDS = 8192
NP_CORE = NPEDS // N_CORES      # 1024
HALF = NP_CORE // 2             # 512
H = 256
E = 64
T = 30
EPS = 1e-5
LEAK = 0.01
MAGIC = 0x5F3759DF
TANH_S_SCALE = 0.88 / float(np.sqrt(4.0 * EPS))

# gate order within a hidden-chunk's 4 psum slots: i, f (fused sigmoid), o, g
GATES = ("i", "f", "o", "g")
GBASE = {"i": 0, "f": H, "g": 2 * H, "o": 3 * H}  # torch row offsets


def _build_program():
    nc = bacc.Bacc(
        "TRN2",
        target_bir_lowering=False,
        debug=False,
        enable_asserts=False,
        num_devices=N_CORES,
    )

    # ---- DRAM I/O ----
    d = {}
    d["LWDR"] = nc.dram_tensor("LWDR", [128, 2048], FP8, kind="ExternalInput")
    d["LW0"] = nc.dram_tensor("LW0", [65, 1024], DT, kind="ExternalInput")
    d["AS"] = nc.dram_tensor("AS", [128, 64], DT, kind="ExternalInput")
    d["SQB"] = nc.dram_tensor("SQB", [128, 64], DT, kind="ExternalInput")
    d["WEMB"] = nc.dram_tensor("WEMB", [33, 64], DT, kind="ExternalInput")
    d["EMBB"] = nc.dram_tensor("EMBB", [64, 1], F32, kind="ExternalInput")
    d["PB0"] = nc.dram_tensor("PB0", [64, 1], F32, kind="ExternalInput")
    d["PB1"] = nc.dram_tensor("PB1", [64, 1], F32, kind="ExternalInput")
    for h in range(2):
        d[f"DEC{h}"] = nc.dram_tensor(f"DEC{h}", [65, HALF], DT,
                                      kind="ExternalInput")
        d[f"TH{h}"] = nc.dram_tensor(f"TH{h}", [128, 1024], FP8,
                                     kind="ExternalInput")
        d[f"HB{h}"] = nc.dram_tensor(f"HB{h}", [128, 1024], DT,
                                     kind="ExternalInput")
        d[f"CB{h}"] = nc.dram_tensor(f"CB{h}", [128, 1024], DT,
                                     kind="ExternalInput")
    out_t = nc.dram_tensor("OUT", [T, 2, NP_CORE], BF16, kind="ExternalOutput")

    with tile.TileContext(nc) as tc:
        with (
            tc.tile_pool(name="weights", bufs=1) as wp,
            tc.tile_pool(name="state", bufs=1) as sp,
            tc.tile_pool(name="acts", bufs=4) as ap_,
            tc.tile_pool(name="dve", bufs=4) as dp,
            tc.tile_pool(name="tail", bufs=2) as tp,
            tc.tile_pool(name="pif", bufs=2, space="PSUM") as pif,
            tc.tile_pool(name="pog", bufs=2, space="PSUM") as pog,
            tc.tile_pool(name="pst", bufs=1, space="PSUM") as pstp,
            tc.tile_pool(name="pemb", bufs=1, space="PSUM") as pembp,
        ):
            # ---- persistent weights ----
            LWDR = wp.tile([128, 8, 2, 128], FP8, tag="LWDR")
            LW0 = wp.tile([65, 1024], DT, tag="LW0")
            AS = wp.tile([128, 64], DT, tag="AS")
            SQB = wp.tile([128, 64], DT, tag="SQB")
            WEMB = wp.tile([33, 64], DT, tag="WEMB")
            EMBB = wp.tile([64, 1], F32, tag="EMBB")
            PB0 = wp.tile([64, 1], F32, tag="PB0")
            PB1 = wp.tile([64, 1], F32, tag="PB1")
            IONE = wp.tile([64, 16], I32, tag="IONE")
            IMAGIC = wp.tile([64, 16], I32, tag="IMAGIC")
            TTB = sp.tile([64, 512], BF16, tag="TTB")
            nc.sync.dma_start(LWDR[:], d["LWDR"].ap())
            nc.sync.dma_start(LW0[:], d["LW0"].ap())
            nc.sync.dma_start(AS[:], d["AS"].ap())
            nc.sync.dma_start(SQB[:], d["SQB"].ap())
            nc.sync.dma_start(WEMB[:], d["WEMB"].ap())
            nc.sync.dma_start(EMBB[:], d["EMBB"].ap())
            nc.sync.dma_start(PB0[:], d["PB0"].ap())
            nc.sync.dma_start(PB1[:], d["PB1"].ap())
            nc.vector.memset(IONE[:], 1)
            nc.vector.memset(IMAGIC[:], MAGIC)
            nc.vector.memset(TTB[:], 0.0)

            # ---- state tiles: [half][parity] ----
            TH = [[sp.tile([128, 2, 512], FP8, name=f"TH_{h}_{p}",
                           tag=f"TH_{h}_{p}")
                   for p in range(2)] for h in range(2)]
            HB = [[sp.tile([128, 1024], DT, name=f"HB_{h}_{p}",
                           tag=f"HB_{h}_{p}")
                   for p in range(2)] for h in range(2)]
            CB = [[sp.tile([128, 1024], DT, name=f"CB_{h}_{p}",
                           tag=f"CB_{h}_{p}")
                   for p in range(2)] for h in range(2)]
            T0 = [[sp.tile([65, HALF], DT, name=f"T0_{h}_{p}",
                           tag=f"T0_{h}_{p}")
                   for p in range(2)] for h in range(2)]
            HQ = [sp.tile([128, 1024], DT, name=f"HQ_{h}", tag=f"HQ_{h}")
                  for h in range(2)]
            HQS = [sp.tile([128, 512], DT, name=f"HQS_{h}", tag=f"HQS_{h}")
                   for h in range(2)]
            TC = [sp.tile([128, 1024], DT, name=f"TC_{h}", tag=f"TC_{h}")
                  for h in range(2)]
            TT = sp.tile([64, 512], F32, tag="TT")
            SB = sp.tile([64, 512], BF16, tag="SB")

            for h in range(2):
                nc.sync.dma_start(TH[h][0][:], d[f"TH{h}"].ap())
                nc.sync.dma_start(HB[h][0][:], d[f"HB{h}"].ap())
                nc.sync.dma_start(CB[h][0][:], d[f"CB{h}"].ap())
                nc.sync.dma_start(T0[h][0][:], d[f"DEC{h}"].ap())
                # parity-1 dec tile needs its ones row pre-set (embed writes
                # only rows 0:64)
                nc.vector.memset(T0[h][1][64:65, :], 1.0)

            def dr_set(h, ch, p):
                """4 DoubleRow matmuls into fresh psum tiles (groups open)."""
                p_if = pif.tile([128, 1024], F32, tag="p_if")
                for j in range(2):
                    k = ch * 4 + j
                    nc.tensor.matmul(p_if[:, j * 512:(j + 1) * 512],
                                     LWDR[:, k, :, :], TH[h][p][:],
                                     start=True, stop=False,
                                     perf_mode=PM.DoubleRow)
                p_o = pog.tile([128, 512], F32, tag="p_og")
                nc.tensor.matmul(p_o[:], LWDR[:, ch * 4 + 2, :, :],
                                 TH[h][p][:], start=True, stop=False,
                                 perf_mode=PM.DoubleRow)
                p_g = pog.tile([128, 512], F32, tag="p_og")
                nc.tensor.matmul(p_g[:], LWDR[:, ch * 4 + 3, :, :],
                                 TH[h][p][:], start=True, stop=False,
                                 perf_mode=PM.DoubleRow)
                return (p_if, p_o, p_g)

            def dec_set(h, ch, p, tiles):
                """4 dec matmuls (K=65, bias via ones row) closing groups."""
                p_if, p_o, p_g = tiles
                for j in range(2):
                    m = (ch * 4 + j) * 128
                    nc.tensor.matmul(p_if[:, j * 512:(j + 1) * 512],
                                     LW0[:, m:m + 128], T0[h][p][:],
                                     start=False, stop=True)
                m = (ch * 4 + 2) * 128
                nc.tensor.matmul(p_o[:], LW0[:, m:m + 128], T0[h][p][:],
                                 start=False, stop=True)
                m = (ch * 4 + 3) * 128
                nc.tensor.matmul(p_g[:], LW0[:, m:m + 128], T0[h][p][:],
                                 start=False, stop=True)

            def acts_cell(h, ch, p, q, tiles, a_o):
                p_if, p_o, p_g = tiles
                a_if = ap_.tile([128, 1024], DT, tag="a_if")
                nc.scalar.activation(a_if[:], p_if[:], AF.Sigmoid)
                nc.scalar.activation(a_o[:, ch * 512:(ch + 1) * 512],
                                     p_o[:], AF.Sigmoid)
                a_g = ap_.tile([128, 512], DT, tag="a_g")
                nc.scalar.activation(a_g[:], p_g[:], AF.Tanh)

                cs = slice(ch * 512, (ch + 1) * 512)
                m1 = dp.tile([128, 512], DT, tag="m1")
                nc.vector.tensor_tensor(m1[:], a_if[:, 512:1024],
                                        CB[h][p][:, cs], OP.mult)
                m2 = dp.tile([128, 512], DT, tag="m2")
                nc.vector.tensor_tensor(m2[:], a_if[:, 0:512], a_g[:],
                                        OP.mult)
                nc.vector.tensor_tensor(CB[h][q][:, cs], m1[:], m2[:],
                                        OP.add)

            def finish_half(h, q, a_o):
                nc.scalar.activation(TC[h][:], CB[h][q][:], AF.Tanh)
                nc.vector.tensor_tensor(HB[h][q][:], a_o[:], TC[h][:],
                                        OP.mult)
                # fp8 copy for next-step gates (DVE; gpsimd fp8-out is slow)
                nc.vector.tensor_copy(TH[h][q][:, :, :], HB[h][q][:])
                nc.gpsimd.tensor_tensor(HQ[h][:], HB[h][q][:], HB[h][q][:],
                                        OP.mult)
                nc.gpsimd.tensor_tensor(HQS[h][:], HQ[h][:, 0:512],
                                        HQ[h][:, 512:1024], OP.add)

            def stats(h, q, st):
                dst = st[h * 32:(h + 1) * 32, :]
                nc.tensor.matmul(dst, AS[:, 0:32], HB[h][q][:, 0:512],
                                 start=True, stop=False)
                nc.tensor.matmul(dst, AS[:, 32:64], HB[h][q][:, 512:1024],
                                 start=False, stop=False)
                nc.tensor.matmul(dst, SQB[:, 0:32], HQS[h][:],
                                 start=False, stop=True)
                # into block-transposed domain
                nc.vector.transpose(TT[h * 32:(h + 1) * 32, :], dst)

            def tail(t, q):
                """Merged-halves LN2 tail in [64,16] block domain."""
                c_num0 = TT[:, 0::32]
                c_num1 = TT[:, 1::32]
                c_mu = TT[:, 2::32]
                c_eh2 = TT[:, 3::32]

                mu2 = dp.tile([64, 16], F32, tag="mu2")
                nc.vector.scalar_tensor_tensor(mu2[:], c_mu, -1.0, c_mu,
                                               OP.mult, OP.mult)
                V = dp.tile([64, 16], F32, tag="V")
                nc.vector.scalar_tensor_tensor(V[:], mu2[:], EPS, c_eh2,
                                               OP.add, OP.add)
                # rsqrt: int bit-trick seed + 1 Newton step
                y = dp.tile([64, 16], F32, tag="y")
                vi = V.bitcast(I32)
                yi = y.bitcast(I32)
                sh = dp.tile([64, 16], I32, tag="sh")
                nc.vector.tensor_tensor(sh[:], vi[:], IONE[:],
                                        OP.arith_shift_right)
                nc.vector.tensor_tensor(yi[:], IMAGIC[:], sh[:], OP.subtract)
                a = dp.tile([64, 16], F32, tag="nra")
                nc.vector.tensor_tensor(a[:], y[:], y[:], OP.mult)
                nc.vector.scalar_tensor_tensor(a[:], a[:], -0.5, V[:],
                                               OP.mult, OP.mult)
                nc.vector.scalar_tensor_tensor(y[:], a[:], 1.5, y[:],
                                               OP.add, OP.mult)

                z0 = dp.tile([64, 16], F32, tag="z0")
                nc.vector.tensor_tensor(z0[:], c_num0, y[:], OP.mult)
                z1 = dp.tile([64, 16], F32, tag="z1")
                nc.vector.tensor_tensor(z1[:], c_num1, y[:], OP.mult)
                nc.scalar.activation(TTB[:, 1::32], z0[:], AF.Sigmoid,
                                     bias=PB0[:])
                nc.scalar.activation(TTB[:, 2::32], z1[:], AF.Sigmoid,
                                     bias=PB1[:])
                e = dp.tile([64, 16], F32, tag="e")
                nc.vector.tensor_tensor(e[:], TTB[:, 1::32], TTB[:, 2::32],
                                        OP.subtract)
                nc.scalar.activation(TTB[:, 0::32], e[:], AF.Tanh,
                                     scale=TANH_S_SCALE)
                # s rows (0,32) for embed + rel rows for DMA
                nc.vector.transpose(SB[0:32, :], TTB[0:32, :])
                nc.vector.transpose(SB[32:64, :], TTB[32:64, :])
                dst = out_t.ap()[t]
                nc.sync.dma_start(dst[:, 0:HALF], SB[1:3, :])
                nc.sync.dma_start(dst[:, HALF:NP_CORE], SB[33:35, :])

            def embed(h, q):
                pe = pembp.tile([64, 512], F32, tag="pe")
                nc.tensor.matmul(pe[:], WEMB[h * 32:h * 32 + 1, :],
                                 SB[h * 32:h * 32 + 1, :],
                                 start=True, stop=True)
                nc.scalar.activation(T0[h][q][0:64, :], pe[:],
                                     AF.Prelu, bias=EMBB[:], alpha=LEAK)

            st = pstp.tile([64, 512], F32, tag="st")
            parked = None
            AO = [None, None]
            for t in range(T):
                p, q = t % 2, (t + 1) % 2
                sets = [(0, 0), (0, 1), (1, 0), (1, 1)]
                if parked is not None:
                    AO[0] = ap_.tile([128, 1024], DT, name="a_o0", tag="a_o0")
                    dec_set(0, 0, p, parked)
                    acts_cell(0, 0, p, q, parked, AO[0])
                    sets = sets[1:]
                for (h, ch) in sets:
                    if ch == 0:
                        AO[h] = ap_.tile([128, 1024], DT, name=f"a_o{h}",
                                         tag=f"a_o{h}")
                    tiles = dr_set(h, ch, p)
                    dec_set(h, ch, p, tiles)
                    acts_cell(h, ch, p, q, tiles, AO[h])
                    if (h, ch) == (0, 1):
                        finish_half(0, q, AO[0])
                    if (h, ch) == (1, 1):
                        finish_half(1, q, AO[1])
                stats(0, q, st)
                stats(1, q, st)
                if t + 1 < T:
                    parked = dr_set(0, 0, q)
                tail(t, q)
                if t + 1 < T:
                    embed(0, q)
                    embed(1, q)

    nc.compile()
    return nc


_NC_CACHE = None


def _get_program():
    global _NC_CACHE
    if _NC_CACHE is None:
        _NC_CACHE = _build_program()
    return _NC_CACHE


def _wemb33(w_emb):
    w = np.zeros((33, 64), np.float32)
    w[0] = w_emb
    w[32] = w_emb
    return np.ascontiguousarray(w)


def _prepare_in_maps(inputs):
    f32 = np.float32
    bf = ml_dtypes.bfloat16
    f8 = ml_dtypes.float8_e4m3
    inp = {k: np.asarray(v, f32) for k, v in inputs.items()}
    W_ih, W_hh = inp["W_ih"], inp["W_hh"]
    bias = (inp["b_ih"] + inp["b_hh"]).astype(f32)

    # chunk k = ch*4 + j over gate order (i, f, o, g), hidden chunk ch
    LWDR = np.zeros((128, 8, 2, 128), f32)
    LW0 = np.zeros((65, 1024), f32)
    for ch in range(2):
        for j, gname in enumerate(GATES):
            k = ch * 4 + j
            rows = slice(GBASE[gname] + ch * 128, GBASE[gname] + ch * 128 + 128)
            LWDR[:, k, 0, :] = W_hh[rows, 0:128].T
            LWDR[:, k, 1, :] = W_hh[rows, 128:256].T
            LW0[0:64, k * 128:(k + 1) * 128] = W_ih[rows, :].T
            LW0[64, k * 128:(k + 1) * 128] = bias[rows]

    emb_W, emb_b = inp["emb_W"], inp["emb_b"]
    g1, b1 = inp["ln1_g"], inp["ln1_b"]
    w_emb = (g1[0] * emb_W[:, 0] - g1[1] * emb_W[:, 1]).astype(f32)
    emb_bp = (emb_b + b1[0] * emb_W[:, 0] + b1[1] * emb_W[:, 1]).astype(f32)

    pos_W, pos_b = inp["pos_W"], inp["pos_b"]
    g2, b2 = inp["ln2_g"], inp["ln2_b"]
    posWp = (pos_W * g2[None, :]).astype(f32)
    pos_bp = (pos_b + b2 @ pos_W.T).astype(f32)
    w1 = posWp.sum(1)
    A = posWp - w1[:, None] / H                            # [2, 256]

    AS = np.zeros((128, 64), f32)
    AS[:, 0], AS[:, 1], AS[:, 2] = A[0, 0:128], A[1, 0:128], 1.0 / H
    AS[:, 32], AS[:, 33], AS[:, 34] = A[0, 128:256], A[1, 128:256], 1.0 / H
    SQB = np.zeros((128, 64), f32)
    SQB[:, 3] = 1.0 / H
    SQB[:, 32 + 3] = 1.0 / H

    lpr = inp["last_pos_rel"]
    e0 = lpr[:, 0] - lpr[:, 1]
    s0 = e0 / np.sqrt(e0 * e0 + 4 * EPS)
    z = s0[:, None] * w_emb[None, :] + emb_bp[None, :]
    dec0 = np.where(z > 0, z, LEAK * z).astype(f32)        # [N, 64]
    dec0T = np.ascontiguousarray(dec0.T)                   # [64, N]

    h0T = np.ascontiguousarray(inp["h0"][0].T)             # [256, N]
    c0T = np.ascontiguousarray(inp["c0"][0].T)

    rep = {
        "LWDR": LWDR.reshape(128, 2048).astype(f8),
        "LW0": LW0.astype(bf),
        "AS": AS.astype(bf),
        "SQB": SQB.astype(bf),
        "WEMB": _wemb33(w_emb).astype(bf),
        "EMBB": np.ascontiguousarray(emb_bp[:, None]).astype(f32),
        "PB0": np.full((64, 1), pos_bp[0], f32),
        "PB1": np.full((64, 1), pos_bp[1], f32),
    }
    in_maps = []
    for c in range(N_CORES):
        m = dict(rep)
        for h in range(2):
            cols = slice(c * NP_CORE + h * HALF, c * NP_CORE + (h + 1) * HALF)
            dec = np.ones((65, HALF), f32)
            dec[0:64] = dec0T[:, cols]
            m[f"DEC{h}"] = dec.astype(bf)
            hb = np.concatenate([h0T[0:128, cols], h0T[128:256, cols]], 1)
            m[f"HB{h}"] = hb.astype(bf)
            m[f"TH{h}"] = hb.astype(f8)
            m[f"CB{h}"] = np.concatenate(
                [c0T[0:128, cols], c0T[128:256, cols]], 1).astype(bf)
        in_maps.append(m)
    return in_maps


def run_on_hw(inputs, trace=False, **kwargs):
    nc = _get_program()
    in_maps = _prepare_in_maps(inputs)
    old_m = nc.m
    nc.m = get_hw_module(nc.m)
    try:
        res = bass_utils.run_bass_kernel_spmd(
            nc, in_maps, core_ids=list(range(N_CORES)), trace=trace, **kwargs)
    finally:
        nc.m = old_m
    out = np.concatenate(
        [np.asarray(r["OUT"], np.float32) for r in res.results], axis=2)
    out = np.ascontiguousarray(out.transpose(0, 2, 1))
    return out.astype(np.float32), res


def kernel(**inputs) -> np.ndarray:
    out, _ = run_on_hw(inputs, trace=False)
    return out
